# revision 2
# baseline (speedup 1.0000x reference)
"""Trainium2 Bass kernel for nn_DifferentiableRollout.

Computes, for B=1024 batched rollouts of T=200 steps:
    x_{t+1} = x_t + DT * ( tanh(concat(x_t, u_t) @ W1 + b1) @ W2 + b2 )
returning all states [B, T+1, SD].

Strategy (8 NeuronCores, data-parallel over batch, 128 rows/core):
  * Per core, the 128 batch rows are split into STREAMS=2 independent
    streams of 64 so the two per-step dependency chains interleave on
    the engines (the T loop is truly sequential; a single chain leaves
    every engine mostly idle).
  * Layout "B": features on partitions, batch on the free dim.
      xu tile [97, W]   rows = [x (64) ; ones (1) ; u (32)]   fp16
      W1aug   [97, 512] rows = [W1_x ; b1 ; W1_u]             fp16
      mm1 (4 chunks of 128 feats) -> psum_h [128, 4*W] packed
      tanh on ScalarE  psum_h -> h [128, 4*W] fp16
      mm2 (4 K-chunks, W2 pre-scaled by DT, zero-padded to 65 cols)
          accumulates into psum_x [65, W]
  * The running state lives IN PSUM for the whole rollout: psum_x is an
    accumulation group that never restarts (row 64 carries a constant
    1.0 that becomes the "ones" row of the next xu).  b2 is handled on
    the way out: xu_{t+1} = fp16(psum_x + (t+1)*DT*b2) via one DVE
    tensor_scalar (psum_x accumulates only matmul deltas).
  * fp16 matmul inputs + fp32 PSUM accumulation keeps the 200-step
    rollout error ~1e-3 relative (bf16 would be ~8e-3).
  * Controls are DMA'd per step straight into xu rows 65:97; states are
    DMA'd out per step from xu rows 0:64 (fp16, cast to f32 on host).
"""

import os
from contextlib import ExitStack

import numpy as np

import concourse.bacc as bacc
import concourse.bass as bass
import concourse.mybir as mybir
import concourse.tile as tile
from concourse.bass_utils import run_bass_kernel_spmd

B, T, SD, CD, H = 1024, 200, 64, 32, 512
DT = 0.1
NCORES = 8
STREAMS = 2
BLOCAL = B // NCORES          # 128 batch rows per core
W = BLOCAL // STREAMS         # batch columns per stream
HCH = H // 128                # 4 feature chunks
KA = SD + 1                   # 65: x rows + ones row
KX = SD + 1 + CD              # 97: rows of xu / W1aug

F16 = mybir.dt.float16
F32 = mybir.dt.float32


def _build_module(t_steps: int = T, streams: int = STREAMS, w: int = W):
    """Build + compile the per-core Bass module (SPMD: same NEFF, 8 cores)."""
    ts = bass.ts
    nc = bacc.Bacc(
        "TRN2",
        target_bir_lowering=False,
        debug=False,
        enable_asserts=False,
        num_devices=NCORES,
    )

    d_w1 = nc.dram_tensor("w1aug", [KX, H], F16, kind="ExternalInput")
    d_w2 = nc.dram_tensor("w2pad", [128, HCH * KA], F16, kind="ExternalInput")
    d_id = nc.dram_tensor("ident", [KA, KA], F16, kind="ExternalInput")
    d_b2 = nc.dram_tensor("b2t", [KA, t_steps + 1], F32, kind="ExternalInput")
    d_x0, d_ct, d_out = [], [], []
    for s in range(streams):
        d_x0.append(nc.dram_tensor(f"x0aug{s}", [KA, w], F16, kind="ExternalInput"))
        d_ct.append(nc.dram_tensor(f"ctrl{s}", [t_steps, CD, w], F16, kind="ExternalInput"))
        d_out.append(nc.dram_tensor(f"out{s}", [t_steps, SD, w], F16, kind="ExternalOutput"))

    with tile.TileContext(nc) as tc, ExitStack() as ctx:
        const = ctx.enter_context(tc.tile_pool(name="const", bufs=1))
        xupool = ctx.enter_context(tc.tile_pool(name="xu", bufs=6))
        hpool = ctx.enter_context(tc.tile_pool(name="h", bufs=2))
        psum = ctx.enter_context(tc.tile_pool(name="psum", bufs=1, space="PSUM"))

        w1_sb = const.tile([KX, H], F16)
        nc.sync.dma_start(w1_sb[:], d_w1.ap()[:])
        w2_sb = const.tile([128, HCH, KA], F16)
        nc.sync.dma_start(w2_sb[:], d_w2.ap().rearrange("p (j c) -> p j c", j=HCH))
        id_sb = const.tile([KA, KA], F16)
        nc.sync.dma_start(id_sb[:], d_id.ap()[:])
        b2_sb = const.tile([KA, t_steps + 1], F32)
        nc.sync.dma_start(b2_sb[:], d_b2.ap()[:])

        psum_x = []
        psum_h = []
        xu = []
        for s in range(streams):
            psum_x.append(psum.tile([KA, w], F32, tag=f"px{s}", name=f"px{s}"))
            psum_h.append(psum.tile([128, HCH * w], F32, tag=f"ph{s}", name=f"ph{s}"))
            t0 = xupool.tile([KX, w], F16, tag=f"xu{s}", name=f"xu{s}")
            nc.sync.dma_start(t0[0:KA, :], d_x0[s].ap()[:])
            nc.sync.dma_start(t0[KA:KX, :], d_ct[s].ap()[0])
            # Seed psum_x with [x0; ones] (exact: identity matmul in fp16).
            nc.tensor.matmul(
                psum_x[s][:], id_sb[:], t0[0:KA, :],
                start=True, stop=False, skip_group_check=True,
            )
            xu.append(t0)

        for t in range(t_steps):
            last_t = t == t_steps - 1
            for s in range(streams):
                # mm1: preactivations for all H features, packed in one bank
                for j in range(HCH):
                    nc.tensor.matmul(
                        psum_h[s][:, ts(j, w)],
                        w1_sb[:, ts(j, 128)],
                        xu[s][:],
                        start=True, stop=True,
                    )
                h_sb = hpool.tile([128, HCH * w], F16, tag=f"h{s}", name=f"h{s}")
                nc.scalar.activation(
                    h_sb[:], psum_h[s][:], mybir.ActivationFunctionType.Tanh
                )
                # mm2: accumulate DT*(h @ W2) into the PSUM-resident state
                for j in range(HCH):
                    nc.tensor.matmul(
                        psum_x[s][:],
                        w2_sb[:, j, :],
                        h_sb[:, ts(j, w)],
                        start=False, stop=last_t and j == HCH - 1,
                        skip_group_check=True,
                    )
                # next xu: fp16(psum_x + (t+1)*DT*b2); row 64 stays 1.0
                xu_n = xupool.tile([KX, w], F16, tag=f"xu{s}", name=f"xu{s}")
                nc.vector.tensor_scalar_add(
                    xu_n[0:KA, :], psum_x[s][:], b2_sb[:, t + 1 : t + 2]
                )
                if not last_t:
                    nc.sync.dma_start(xu_n[KA:KX, :], d_ct[s].ap()[t + 1])
                nc.sync.dma_start(d_out[s].ap()[t], xu_n[0:SD, :])
                xu[s] = xu_n

    nc.compile()
    return nc


_CACHE: dict = {}


def _get_module():
    if "nc" not in _CACHE:
        _CACHE["nc"] = _build_module()
    return _CACHE["nc"]


def _prep_inputs(x0, controls, W1, b1, W2, b2):
    """Host-side prep: shard, transpose, augment, cast. Returns in_maps."""
    f16 = np.float16
    W1 = np.asarray(W1, np.float32)
    b1 = np.asarray(b1, np.float32)
    W2 = np.asarray(W2, np.float32)
    b2 = np.asarray(b2, np.float32)
    x0 = np.asarray(x0, np.float32)
    controls = np.asarray(controls, np.float32)

    w1aug = np.concatenate([W1[:SD], b1[None, :], W1[SD:]], axis=0).astype(f16)
    w2pad = np.zeros((H, KA), np.float32)
    w2pad[:, :SD] = DT * W2
    w2pad = w2pad.reshape(HCH, 128, KA).transpose(1, 0, 2).reshape(128, HCH * KA)
    w2pad = w2pad.astype(f16)
    ident = np.eye(KA, dtype=f16)
    b2t = np.zeros((KA, T + 1), np.float32)
    b2t[:SD] = np.outer(DT * b2, np.arange(T + 1, dtype=np.float32))

    x0T = x0.T.astype(f16)                       # [SD, B]
    ctrlT = controls.transpose(1, 2, 0).astype(f16)  # [T, CD, B]

    in_maps = []
    for c in range(NCORES):
        m = {"w1aug": w1aug, "w2pad": w2pad, "ident": ident, "b2t": b2t}
        for s in range(STREAMS):
            lo = c * BLOCAL + s * W
            cols = slice(lo, lo + W)
            x0aug = np.concatenate(
                [x0T[:, cols], np.ones((1, W), f16)], axis=0
            ).astype(f16)
            m[f"x0aug{s}"] = x0aug
            m[f"ctrl{s}"] = np.ascontiguousarray(ctrlT[:, :, cols])
        in_maps.append(m)
    return in_maps


def kernel(x0, controls, W1, b1, W2, b2):
    nc = _get_module()
    in_maps = _prep_inputs(x0, controls, W1, b1, W2, b2)
    res = run_bass_kernel_spmd(nc, in_maps, core_ids=list(range(NCORES)))

    states = np.empty((B, T + 1, SD), np.float32)
    states[:, 0, :] = np.asarray(x0, np.float32)
    for c in range(NCORES):
        for s in range(STREAMS):
            lo = c * BLOCAL + s * W
            out = np.asarray(res.results[c][f"out{s}"], np.float16)  # [T, SD, W]
            states[lo : lo + W, 1:, :] = out.transpose(2, 0, 1).astype(np.float32)
    return states


# revision 3
# speedup vs baseline: 1.5814x; 1.5814x over previous
"""Trainium2 Bass kernel for nn_DifferentiableRollout.

Computes, for B=1024 batched rollouts of T=200 steps:
    x_{t+1} = x_t + DT * ( tanh(concat(x_t, u_t) @ W1 + b1) @ W2 + b2 )
returning all states [B, T+1, SD].

Strategy (8 NeuronCores, data-parallel over batch, 128 rows/core):
  * Per core, the 128 batch rows are split into STREAMS=2 independent
    streams of 64 so the two per-step dependency chains interleave on
    the engines (the T loop is truly sequential; a single chain leaves
    every engine mostly idle).
  * Layout "B": features on partitions, batch on the free dim.  One
    persistent SBUF "mega" tile per stream [97, (T+1)*W] fp16 holds the
    whole rollout: slot t columns [t*W,(t+1)*W) carry
        rows 0:64  x_t      (written by the per-step DVE op)
        row  64    ones     (comes from PSUM row 64, always 1.0)
        rows 65:97 u_t      (ALL controls preloaded by one DMA)
    so matmul-1's moving operand is just a slice — no per-step copies
    or DMAs at all.
  * Per step per stream:
        mm1 x4:  psum_h[:, j*W:(j+1)*W] = W1aug_j.T @ mega[:, slot t]
        tanh:    h = tanh(psum_h)            (ScalarE, psum -> sbuf fp16)
        mm2 x4:  psum_x += W2pad_j.T @ h_j   (accumulates forever)
        DVE:     mega[0:65, slot t+1] = fp16(psum_x + (t+1)*DT*b2)
  * The running state lives IN PSUM for the whole rollout: psum_x is an
    accumulation group that never restarts.  W1aug embeds b1 via the
    ones row; W2 is pre-scaled by DT and zero-padded to 65 columns so
    PSUM row 64 stays exactly 1.0; b2 is applied on the way out via the
    per-partition scalar (t+1)*DT*b2 (psum_x holds only matmul deltas).
  * fp16 matmul inputs + fp32 PSUM accumulation keeps the 200-step
    rollout error ~1e-3 relative (bf16 would be ~8e-3).
  * States leave as fp16 in 4 chunked DMAs that the scheduler overlaps
    with the rollout; the host casts to f32 (t=0 is copied exactly).
"""

from contextlib import ExitStack

import numpy as np

import concourse.bacc as bacc
import concourse.bass as bass
import concourse.mybir as mybir
import concourse.tile as tile
from concourse.bass_utils import run_bass_kernel_spmd

B, T, SD, CD, H = 1024, 200, 64, 32, 512
DT = 0.1
NCORES = 8
STREAMS = 2
BLOCAL = B // NCORES          # 128 batch rows per core
W = BLOCAL // STREAMS         # batch columns per stream
HCH = H // 128                # 4 feature chunks
KA = SD + 1                   # 65: x rows + ones row
KX = SD + 1 + CD              # 97: rows of xu / W1aug
OUT_CHUNKS = 4                # states leave in this many DMAs per stream

F16 = mybir.dt.float16
F32 = mybir.dt.float32


def _build_module(t_steps: int = T, streams: int = STREAMS, w: int = W):
    """Build + compile the per-core Bass module (SPMD: same NEFF, 8 cores)."""
    ts = bass.ts
    nc = bacc.Bacc(
        "TRN2",
        target_bir_lowering=False,
        debug=False,
        enable_asserts=False,
        num_devices=NCORES,
    )

    d_w1 = nc.dram_tensor("w1aug", [KX, H], F16, kind="ExternalInput")
    d_w2 = nc.dram_tensor("w2pad", [128, HCH * KA], F16, kind="ExternalInput")
    d_id = nc.dram_tensor("ident", [KA, KA], F16, kind="ExternalInput")
    d_b2 = nc.dram_tensor("b2t", [KA, t_steps + 1], F32, kind="ExternalInput")
    d_x0, d_ct, d_out = [], [], []
    for s in range(streams):
        d_x0.append(nc.dram_tensor(f"x0aug{s}", [KA, w], F16, kind="ExternalInput"))
        d_ct.append(nc.dram_tensor(f"ctrl{s}", [CD, t_steps * w], F16, kind="ExternalInput"))
        d_out.append(
            nc.dram_tensor(f"out{s}", [SD, (t_steps + 1) * w], F16, kind="ExternalOutput")
        )

    with tile.TileContext(nc) as tc, ExitStack() as ctx:
        const = ctx.enter_context(tc.tile_pool(name="const", bufs=1))
        hpool = ctx.enter_context(tc.tile_pool(name="h", bufs=2))
        psum = ctx.enter_context(tc.tile_pool(name="psum", bufs=1, space="PSUM"))

        w1_sb = const.tile([KX, H], F16)
        nc.sync.dma_start(w1_sb[:], d_w1.ap()[:])
        w2_sb = const.tile([128, HCH, KA], F16)
        nc.sync.dma_start(w2_sb[:], d_w2.ap().rearrange("p (j c) -> p j c", j=HCH))
        id_sb = const.tile([KA, KA], F16)
        nc.sync.dma_start(id_sb[:], d_id.ap()[:])
        b2_sb = const.tile([KA, t_steps + 1], F32)
        nc.sync.dma_start(b2_sb[:], d_b2.ap()[:])

        psum_x, psum_h, mega = [], [], []
        for s in range(streams):
            psum_x.append(psum.tile([KA, w], F32, tag=f"px{s}", name=f"px{s}"))
            psum_h.append(psum.tile([128, HCH * w], F32, tag=f"ph{s}", name=f"ph{s}"))
            m = const.tile([KX, (t_steps + 1) * w], F16, name=f"mega{s}")
            nc.sync.dma_start(m[KA:KX, 0 : t_steps * w], d_ct[s].ap()[:])
            nc.sync.dma_start(m[0:KA, 0:w], d_x0[s].ap()[:])
            # Seed psum_x with [x0; ones] (exact: identity matmul in fp16).
            nc.tensor.matmul(
                psum_x[s][:], id_sb[:], m[0:KA, 0:w],
                start=True, stop=False, skip_group_check=True,
            )
            mega.append(m)

        for t in range(t_steps):
            last_t = t == t_steps - 1
            for s in range(streams):
                # mm1: preactivations for all H features, packed in one bank
                for j in range(HCH):
                    nc.tensor.matmul(
                        psum_h[s][:, ts(j, w)],
                        w1_sb[:, ts(j, 128)],
                        mega[s][:, ts(t, w)],
                        start=True, stop=True,
                    )
                h_sb = hpool.tile([128, HCH * w], F16, tag=f"h{s}", name=f"h{s}")
                nc.scalar.activation(
                    h_sb[:], psum_h[s][:], mybir.ActivationFunctionType.Tanh
                )
                # mm2: accumulate DT*(h @ W2) into the PSUM-resident state
                for j in range(HCH):
                    nc.tensor.matmul(
                        psum_x[s][:],
                        w2_sb[:, j, :],
                        h_sb[:, ts(j, w)],
                        start=False, stop=last_t and j == HCH - 1,
                        skip_group_check=True,
                    )
                # next x slot: fp16(psum_x + (t+1)*DT*b2); row 64 stays 1.0
                nc.vector.tensor_scalar_add(
                    mega[s][0:KA, ts(t + 1, w)], psum_x[s][:], b2_sb[:, t + 1 : t + 2]
                )

        # states out, chunked so DMA overlaps the tail of the rollout
        bounds = np.linspace(0, t_steps + 1, OUT_CHUNKS + 1).astype(int)
        for s in range(streams):
            for k in range(OUT_CHUNKS):
                a, b = int(bounds[k]), int(bounds[k + 1])
                if a == b:
                    continue
                nc.sync.dma_start(
                    d_out[s].ap()[:, a * w : b * w], mega[s][0:SD, a * w : b * w]
                )

    nc.compile()
    return nc


_CACHE: dict = {}


def _get_module():
    if "nc" not in _CACHE:
        _CACHE["nc"] = _build_module()
    return _CACHE["nc"]


def _prep_inputs(x0, controls, W1, b1, W2, b2):
    """Host-side prep: shard, transpose, augment, cast. Returns in_maps."""
    f16 = np.float16
    W1 = np.asarray(W1, np.float32)
    b1 = np.asarray(b1, np.float32)
    W2 = np.asarray(W2, np.float32)
    b2 = np.asarray(b2, np.float32)
    x0 = np.asarray(x0, np.float32)
    controls = np.asarray(controls, np.float32)

    w1aug = np.concatenate([W1[:SD], b1[None, :], W1[SD:]], axis=0).astype(f16)
    w2pad = np.zeros((H, KA), np.float32)
    w2pad[:, :SD] = DT * W2
    w2pad = w2pad.reshape(HCH, 128, KA).transpose(1, 0, 2).reshape(128, HCH * KA)
    w2pad = w2pad.astype(f16)
    ident = np.eye(KA, dtype=f16)
    b2t = np.zeros((KA, T + 1), np.float32)
    b2t[:SD] = np.outer(DT * b2, np.arange(T + 1, dtype=np.float32))

    x0T = x0.T.astype(f16)                           # [SD, B]
    ctrlT = controls.transpose(1, 2, 0).astype(f16)  # [T, CD, B]

    in_maps = []
    for c in range(NCORES):
        m = {"w1aug": w1aug, "w2pad": w2pad, "ident": ident, "b2t": b2t}
        for s in range(STREAMS):
            lo = c * BLOCAL + s * W
            cols = slice(lo, lo + W)
            m[f"x0aug{s}"] = np.concatenate(
                [x0T[:, cols], np.ones((1, W), f16)], axis=0
            ).astype(f16)
            # [CD, T*W]: slot t columns hold u_t
            m[f"ctrl{s}"] = np.ascontiguousarray(
                ctrlT[:, :, cols].transpose(1, 0, 2)
            ).reshape(CD, T * W)
        in_maps.append(m)
    return in_maps


def kernel(x0, controls, W1, b1, W2, b2):
    nc = _get_module()
    in_maps = _prep_inputs(x0, controls, W1, b1, W2, b2)
    res = run_bass_kernel_spmd(nc, in_maps, core_ids=list(range(NCORES)))

    states = np.empty((B, T + 1, SD), np.float32)
    for c in range(NCORES):
        for s in range(STREAMS):
            lo = c * BLOCAL + s * W
            out = np.asarray(res.results[c][f"out{s}"], np.float16)
            out = out.reshape(SD, T + 1, W).transpose(2, 1, 0)  # [W, T+1, SD]
            states[lo : lo + W] = out.astype(np.float32)
    states[:, 0, :] = np.asarray(x0, np.float32)
    return states


# revision 5
# speedup vs baseline: 1.5821x; 1.0004x over previous
"""Trainium2 Bass kernel for nn_DifferentiableRollout.

Computes, for B=1024 batched rollouts of T=200 steps:
    x_{t+1} = x_t + DT * ( tanh(concat(x_t, u_t) @ W1 + b1) @ W2 + b2 )
returning all states [B, T+1, SD].

Strategy (8 NeuronCores, data-parallel over batch, 128 rows/core):
  * Per core, the 128 batch rows are split into STREAMS=2 independent
    streams of 64 so the two per-step dependency chains interleave on
    the engines (the T loop is truly sequential; a single chain leaves
    every engine mostly idle).
  * Layout "B": features on partitions, batch on the free dim.  One
    persistent SBUF "mega" tile per stream [97, (T+1)*W] fp16 holds the
    whole rollout: slot t columns [t*W,(t+1)*W) carry
        rows 0:64  x_t      (written by the per-step DVE op)
        row  64    ones     (comes from PSUM row 64, always 1.0)
        rows 65:97 u_t      (ALL controls preloaded by one DMA)
    so matmul-1's moving operand is just a slice — no per-step copies
    or DMAs at all.
  * Per step per stream:
        mm1 x4:  psum_h[:, j*W:(j+1)*W] = W1aug_j.T @ mega[:, slot t]
        tanh:    h = tanh(psum_h)            (ScalarE, psum -> sbuf fp16)
        mm2 x4:  psum_x += W2pad_j.T @ h_j   (accumulates forever)
        DVE:     mega[0:65, slot t+1] = fp16(psum_x + (t+1)*DT*b2)
  * The running state lives IN PSUM for the whole rollout: psum_x is an
    accumulation group that never restarts.  W1aug embeds b1 via the
    ones row; W2 is pre-scaled by DT and zero-padded to 65 columns so
    PSUM row 64 stays exactly 1.0; b2 is applied on the way out via the
    per-partition scalar (t+1)*DT*b2 (psum_x holds only matmul deltas).
  * fp16 matmul inputs + fp32 PSUM accumulation keeps the 200-step
    rollout error ~1e-3 relative (bf16 would be ~8e-3).
  * States leave as fp16 in 4 chunked DMAs that the scheduler overlaps
    with the rollout; the host casts to f32 (t=0 is copied exactly).
"""

from contextlib import ExitStack

import numpy as np

import concourse.bacc as bacc
import concourse.bass as bass
import concourse.mybir as mybir
import concourse.tile as tile
from concourse.bass_utils import run_bass_kernel_spmd

B, T, SD, CD, H = 1024, 200, 64, 32, 512
DT = 0.1
NCORES = 8
STREAMS = 2
BLOCAL = B // NCORES          # 128 batch rows per core
W = BLOCAL // STREAMS         # batch columns per stream
HCH = H // 128                # 4 feature chunks
KA = SD + 1                   # 65: x rows + ones row
KX = SD + 1 + CD              # 97: rows of xu / W1aug
OUT_CHUNKS = 8                # states leave in this many DMAs per stream

F16 = mybir.dt.float16
F32 = mybir.dt.float32


def _build_module(t_steps: int = T, streams: int = STREAMS, w: int = W):
    """Build + compile the per-core Bass module (SPMD: same NEFF, 8 cores)."""
    ts = bass.ts
    nc = bacc.Bacc(
        "TRN2",
        target_bir_lowering=False,
        debug=False,
        enable_asserts=False,
        num_devices=NCORES,
    )

    d_w1 = nc.dram_tensor("w1aug", [KX, H], F16, kind="ExternalInput")
    d_w2 = nc.dram_tensor("w2pad", [128, HCH * KA], F16, kind="ExternalInput")
    d_id = nc.dram_tensor("ident", [KA, KA], F16, kind="ExternalInput")
    d_b2 = nc.dram_tensor("b2t", [KA, t_steps + 1], F32, kind="ExternalInput")
    d_x0, d_ct, d_out = [], [], []
    for s in range(streams):
        d_x0.append(nc.dram_tensor(f"x0aug{s}", [KA, w], F16, kind="ExternalInput"))
        d_ct.append(nc.dram_tensor(f"ctrl{s}", [CD, t_steps * w], F16, kind="ExternalInput"))
        d_out.append(
            nc.dram_tensor(f"out{s}", [SD, (t_steps + 1) * w], F16, kind="ExternalOutput")
        )

    with tile.TileContext(nc) as tc, ExitStack() as ctx:
        const = ctx.enter_context(tc.tile_pool(name="const", bufs=1))
        hpool = ctx.enter_context(tc.tile_pool(name="h", bufs=2))
        psum = ctx.enter_context(tc.tile_pool(name="psum", bufs=1, space="PSUM"))

        w1_sb = const.tile([KX, H], F16)
        nc.sync.dma_start(w1_sb[:], d_w1.ap()[:])
        w2_sb = const.tile([128, HCH, KA], F16)
        nc.sync.dma_start(w2_sb[:], d_w2.ap().rearrange("p (j c) -> p j c", j=HCH))
        id_sb = const.tile([KA, KA], F16)
        nc.sync.dma_start(id_sb[:], d_id.ap()[:])
        b2_sb = const.tile([KA, t_steps + 1], F32)
        nc.sync.dma_start(b2_sb[:], d_b2.ap()[:])

        psum_x, psum_h, mega = [], [], []
        for s in range(streams):
            psum_x.append(psum.tile([KA, w], F32, tag=f"px{s}", name=f"px{s}"))
            psum_h.append(psum.tile([128, HCH * w], F32, tag=f"ph{s}", name=f"ph{s}"))
            m = const.tile([KX, (t_steps + 1) * w], F16, name=f"mega{s}")
            # chunked so the first slots land fast and step 0 starts early
            cb = np.linspace(0, t_steps, 5).astype(int)
            for k in range(len(cb) - 1):
                a, b = int(cb[k]), int(cb[k + 1])
                if a == b:
                    continue
                nc.sync.dma_start(
                    m[KA:KX, a * w : b * w], d_ct[s].ap()[:, a * w : b * w]
                )
            nc.sync.dma_start(m[0:KA, 0:w], d_x0[s].ap()[:])
            # Seed psum_x with [x0; ones] (exact: identity matmul in fp16).
            nc.tensor.matmul(
                psum_x[s][:], id_sb[:], m[0:KA, 0:w],
                start=True, stop=False, skip_group_check=True,
            )
            mega.append(m)

        for t in range(t_steps):
            last_t = t == t_steps - 1
            for s in range(streams):
                # mm1: preactivations for all H features, packed in one bank
                for j in range(HCH):
                    nc.tensor.matmul(
                        psum_h[s][:, ts(j, w)],
                        w1_sb[:, ts(j, 128)],
                        mega[s][:, ts(t, w)],
                        start=True, stop=True,
                    )
                h_sb = hpool.tile([128, HCH * w], F16, tag=f"h{s}", name=f"h{s}")
                nc.scalar.activation(
                    h_sb[:], psum_h[s][:], mybir.ActivationFunctionType.Tanh
                )
                # mm2: accumulate DT*(h @ W2) into the PSUM-resident state
                for j in range(HCH):
                    nc.tensor.matmul(
                        psum_x[s][:],
                        w2_sb[:, j, :],
                        h_sb[:, ts(j, w)],
                        start=False, stop=last_t and j == HCH - 1,
                        skip_group_check=True,
                    )
                # next x slot: fp16(psum_x + (t+1)*DT*b2); row 64 stays 1.0
                nc.vector.tensor_scalar_add(
                    mega[s][0:KA, ts(t + 1, w)], psum_x[s][:], b2_sb[:, t + 1 : t + 2]
                )

        # states out, chunked so DMA overlaps the tail of the rollout
        bounds = np.linspace(0, t_steps + 1, OUT_CHUNKS + 1).astype(int)
        for s in range(streams):
            for k in range(OUT_CHUNKS):
                a, b = int(bounds[k]), int(bounds[k + 1])
                if a == b:
                    continue
                nc.sync.dma_start(
                    d_out[s].ap()[:, a * w : b * w], mega[s][0:SD, a * w : b * w]
                )

    nc.compile()
    return nc


_CACHE: dict = {}


def _get_module():
    if "nc" not in _CACHE:
        _CACHE["nc"] = _build_module()
    return _CACHE["nc"]


def _prep_inputs(x0, controls, W1, b1, W2, b2):
    """Host-side prep: shard, transpose, augment, cast. Returns in_maps."""
    f16 = np.float16
    W1 = np.asarray(W1, np.float32)
    b1 = np.asarray(b1, np.float32)
    W2 = np.asarray(W2, np.float32)
    b2 = np.asarray(b2, np.float32)
    x0 = np.asarray(x0, np.float32)
    controls = np.asarray(controls, np.float32)

    w1aug = np.concatenate([W1[:SD], b1[None, :], W1[SD:]], axis=0).astype(f16)
    w2pad = np.zeros((H, KA), np.float32)
    w2pad[:, :SD] = DT * W2
    w2pad = w2pad.reshape(HCH, 128, KA).transpose(1, 0, 2).reshape(128, HCH * KA)
    w2pad = w2pad.astype(f16)
    ident = np.eye(KA, dtype=f16)
    b2t = np.zeros((KA, T + 1), np.float32)
    b2t[:SD] = np.outer(DT * b2, np.arange(T + 1, dtype=np.float32))

    x0T = x0.T.astype(f16)                           # [SD, B]
    ctrlT = controls.transpose(1, 2, 0).astype(f16)  # [T, CD, B]

    in_maps = []
    for c in range(NCORES):
        m = {"w1aug": w1aug, "w2pad": w2pad, "ident": ident, "b2t": b2t}
        for s in range(STREAMS):
            lo = c * BLOCAL + s * W
            cols = slice(lo, lo + W)
            m[f"x0aug{s}"] = np.concatenate(
                [x0T[:, cols], np.ones((1, W), f16)], axis=0
            ).astype(f16)
            # [CD, T*W]: slot t columns hold u_t
            m[f"ctrl{s}"] = np.ascontiguousarray(
                ctrlT[:, :, cols].transpose(1, 0, 2)
            ).reshape(CD, T * W)
        in_maps.append(m)
    return in_maps


def kernel(x0, controls, W1, b1, W2, b2):
    nc = _get_module()
    in_maps = _prep_inputs(x0, controls, W1, b1, W2, b2)
    res = run_bass_kernel_spmd(nc, in_maps, core_ids=list(range(NCORES)))

    states = np.empty((B, T + 1, SD), np.float32)
    for c in range(NCORES):
        for s in range(STREAMS):
            lo = c * BLOCAL + s * W
            out = np.asarray(res.results[c][f"out{s}"], np.float16)
            out = out.reshape(SD, T + 1, W).transpose(2, 1, 0)  # [W, T+1, SD]
            states[lo : lo + W] = out.astype(np.float32)
    states[:, 0, :] = np.asarray(x0, np.float32)
    return states


# revision 7
# speedup vs baseline: 1.8820x; 1.1896x over previous
"""Trainium2 Bass kernel for nn_DifferentiableRollout.

Computes, for B=1024 batched rollouts of T=200 steps:
    x_{t+1} = x_t + DT * ( tanh(concat(x_t, u_t) @ W1 + b1) @ W2 + b2 )
returning all states [B, T+1, SD].

Strategy (8 NeuronCores, data-parallel over batch, 128 rows/core):
  * Per core, the 128 batch rows are split into STREAMS=2 independent
    streams of 64 so the two per-step dependency chains interleave on
    the engines (the T loop is truly sequential; a single chain leaves
    every engine mostly idle).
  * Layout "B": features on partitions, batch on the free dim.  One
    persistent SBUF "mega" tile per stream [97, (T+1)*W] fp16 holds the
    whole rollout: slot t columns [t*W,(t+1)*W) carry
        rows 0:64  x_t      (written by the per-step DVE op)
        row  64    ones     (comes from PSUM row 64, always 1.0)
        rows 65:97 u_t      (ALL controls preloaded by one DMA)
    so matmul-1's moving operand is just a slice — no per-step copies
    or DMAs at all.
  * Per step per stream:
        mm1 x4:  psum_h[:, j*W:(j+1)*W] = W1aug_j.T @ mega[:, slot t]
        tanh:    h = tanh(psum_h)            (ScalarE, psum -> sbuf fp16)
        mm2 x4:  psum_x += W2pad_j.T @ h_j   (accumulates forever)
        DVE:     mega[0:65, slot t+1] = fp16(psum_x + (t+1)*DT*b2)
  * The running state lives IN PSUM for the whole rollout: psum_x is an
    accumulation group that never restarts.  W1aug embeds b1 via the
    ones row; W2 is pre-scaled by DT and zero-padded to 65 columns so
    PSUM row 64 stays exactly 1.0; b2 is applied on the way out via the
    per-partition scalar (t+1)*DT*b2 (psum_x holds only matmul deltas).
  * fp16 matmul inputs + fp32 PSUM accumulation keeps the 200-step
    rollout error ~1e-3 relative (bf16 would be ~8e-3).
  * States leave as fp16 in 4 chunked DMAs that the scheduler overlaps
    with the rollout; the host casts to f32 (t=0 is copied exactly).
"""

from contextlib import ExitStack

import numpy as np

import concourse.bacc as bacc
import concourse.bass as bass
import concourse.mybir as mybir
import concourse.tile as tile
from concourse.bass_utils import run_bass_kernel_spmd

B, T, SD, CD, H = 1024, 200, 64, 32, 512
DT = 0.1
NCORES = 8
STREAMS = 2
BLOCAL = B // NCORES          # 128 batch rows per core
W = BLOCAL // STREAMS         # batch columns per stream
HCH = H // 128                # 4 feature chunks
KA = SD + 1                   # 65: x rows + ones row
KX = SD + 1 + CD              # 97: rows of xu / W1aug
OUT_CHUNKS = 8                # states leave in this many DMAs per stream

F16 = mybir.dt.float16
F32 = mybir.dt.float32


def _build_module(t_steps: int = T, streams: int = STREAMS, w: int = W):
    """Build + compile the per-core Bass module (SPMD: same NEFF, 8 cores)."""
    ts = bass.ts
    nc = bacc.Bacc(
        "TRN2",
        target_bir_lowering=False,
        debug=False,
        enable_asserts=False,
        num_devices=NCORES,
    )

    d_w1 = nc.dram_tensor("w1aug", [KX, H], F16, kind="ExternalInput")
    d_w2 = nc.dram_tensor("w2pad", [128, HCH * KA], F16, kind="ExternalInput")
    d_id = nc.dram_tensor("ident", [KA, KA], F16, kind="ExternalInput")
    d_b2 = nc.dram_tensor("b2t", [KA, t_steps + 1], F32, kind="ExternalInput")
    d_x0, d_ct, d_out = [], [], []
    for s in range(streams):
        d_x0.append(nc.dram_tensor(f"x0aug{s}", [KA, w], F16, kind="ExternalInput"))
        d_ct.append(nc.dram_tensor(f"ctrl{s}", [CD, t_steps * w], F16, kind="ExternalInput"))
        d_out.append(
            nc.dram_tensor(f"out{s}", [SD, (t_steps + 1) * w], F16, kind="ExternalOutput")
        )

    with tile.TileContext(nc) as tc, ExitStack() as ctx:
        const = ctx.enter_context(tc.tile_pool(name="const", bufs=1))
        hpool = ctx.enter_context(tc.tile_pool(name="h", bufs=2))
        psum = ctx.enter_context(tc.tile_pool(name="psum", bufs=1, space="PSUM"))

        w1_sb = const.tile([KX, H], F16)
        nc.sync.dma_start(w1_sb[:], d_w1.ap()[:])
        w2_sb = const.tile([128, HCH, KA], F16)
        nc.sync.dma_start(w2_sb[:], d_w2.ap().rearrange("p (j c) -> p j c", j=HCH))
        id_sb = const.tile([KA, KA], F16)
        nc.sync.dma_start(id_sb[:], d_id.ap()[:])
        b2_sb = const.tile([KA, t_steps + 1], F32)
        nc.sync.dma_start(b2_sb[:], d_b2.ap()[:])

        # Warm-up during the control-preload DMAs: ~3.5us of dummy matmuls
        # lifts the PE HAM clock gate to 2.4 GHz, and a dummy tanh pulls the
        # ACT table load (~2.7us) off the first step's critical path.
        warm_ps = psum.tile([KA, KA], F32, tag="warm", name="warm_ps")
        warm_sb = const.tile([KA, KA], F16, name="warm_sb")
        for _ in range(4):
            nc.tensor.matmul(warm_ps[:], id_sb[:], id_sb[:], start=True, stop=True)
        nc.scalar.activation(
            warm_sb[:], warm_ps[:], mybir.ActivationFunctionType.Tanh
        )

        psum_x, psum_h, mega = [], [], []
        for s in range(streams):
            psum_x.append(psum.tile([KA, w], F32, tag=f"px{s}", name=f"px{s}"))
            psum_h.append(psum.tile([128, HCH * w], F32, tag=f"ph{s}", name=f"ph{s}"))
            m = const.tile([KX, (t_steps + 1) * w], F16, name=f"mega{s}")
            # chunked so the first slots land fast and step 0 starts early
            cb = np.linspace(0, t_steps, 5).astype(int)
            for k in range(len(cb) - 1):
                a, b = int(cb[k]), int(cb[k + 1])
                if a == b:
                    continue
                nc.sync.dma_start(
                    m[KA:KX, a * w : b * w], d_ct[s].ap()[:, a * w : b * w]
                )
            nc.sync.dma_start(m[0:KA, 0:w], d_x0[s].ap()[:])
            # Seed psum_x with [x0; ones] (exact: identity matmul in fp16).
            nc.tensor.matmul(
                psum_x[s][:], id_sb[:], m[0:KA, 0:w],
                start=True, stop=False, skip_group_check=True,
            )
            mega.append(m)

        for t in range(t_steps):
            last_t = t == t_steps - 1
            for s in range(streams):
                # mm1: preactivations for all H features, packed in one bank
                for j in range(HCH):
                    nc.tensor.matmul(
                        psum_h[s][:, ts(j, w)],
                        w1_sb[:, ts(j, 128)],
                        mega[s][:, ts(t, w)],
                        start=True, stop=True,
                    )
                h_sb = hpool.tile([128, HCH * w], F16, tag=f"h{s}", name=f"h{s}")
                nc.scalar.activation(
                    h_sb[:], psum_h[s][:], mybir.ActivationFunctionType.Tanh
                )
                # mm2: accumulate DT*(h @ W2) into the PSUM-resident state
                for j in range(HCH):
                    nc.tensor.matmul(
                        psum_x[s][:],
                        w2_sb[:, j, :],
                        h_sb[:, ts(j, w)],
                        start=False, stop=last_t and j == HCH - 1,
                        skip_group_check=True,
                    )
                # next x slot: fp16(psum_x + (t+1)*DT*b2); row 64 stays 1.0
                nc.vector.tensor_scalar_add(
                    mega[s][0:KA, ts(t + 1, w)], psum_x[s][:], b2_sb[:, t + 1 : t + 2]
                )

        # states out, chunked so DMA overlaps the tail of the rollout
        bounds = np.linspace(0, t_steps + 1, OUT_CHUNKS + 1).astype(int)
        for s in range(streams):
            for k in range(OUT_CHUNKS):
                a, b = int(bounds[k]), int(bounds[k + 1])
                if a == b:
                    continue
                nc.sync.dma_start(
                    d_out[s].ap()[:, a * w : b * w], mega[s][0:SD, a * w : b * w]
                )

    nc.compile()
    return nc


_CACHE: dict = {}


def _get_module():
    if "nc" not in _CACHE:
        _CACHE["nc"] = _build_module()
    return _CACHE["nc"]


def _prep_inputs(x0, controls, W1, b1, W2, b2):
    """Host-side prep: shard, transpose, augment, cast. Returns in_maps."""
    f16 = np.float16
    W1 = np.asarray(W1, np.float32)
    b1 = np.asarray(b1, np.float32)
    W2 = np.asarray(W2, np.float32)
    b2 = np.asarray(b2, np.float32)
    x0 = np.asarray(x0, np.float32)
    controls = np.asarray(controls, np.float32)

    w1aug = np.concatenate([W1[:SD], b1[None, :], W1[SD:]], axis=0).astype(f16)
    w2pad = np.zeros((H, KA), np.float32)
    w2pad[:, :SD] = DT * W2
    w2pad = w2pad.reshape(HCH, 128, KA).transpose(1, 0, 2).reshape(128, HCH * KA)
    w2pad = w2pad.astype(f16)
    ident = np.eye(KA, dtype=f16)
    b2t = np.zeros((KA, T + 1), np.float32)
    b2t[:SD] = np.outer(DT * b2, np.arange(T + 1, dtype=np.float32))

    x0T = x0.T.astype(f16)                           # [SD, B]
    ctrlT = controls.transpose(1, 2, 0).astype(f16)  # [T, CD, B]

    in_maps = []
    for c in range(NCORES):
        m = {"w1aug": w1aug, "w2pad": w2pad, "ident": ident, "b2t": b2t}
        for s in range(STREAMS):
            lo = c * BLOCAL + s * W
            cols = slice(lo, lo + W)
            m[f"x0aug{s}"] = np.concatenate(
                [x0T[:, cols], np.ones((1, W), f16)], axis=0
            ).astype(f16)
            # [CD, T*W]: slot t columns hold u_t
            m[f"ctrl{s}"] = np.ascontiguousarray(
                ctrlT[:, :, cols].transpose(1, 0, 2)
            ).reshape(CD, T * W)
        in_maps.append(m)
    return in_maps


def kernel(x0, controls, W1, b1, W2, b2):
    nc = _get_module()
    in_maps = _prep_inputs(x0, controls, W1, b1, W2, b2)
    res = run_bass_kernel_spmd(nc, in_maps, core_ids=list(range(NCORES)))

    states = np.empty((B, T + 1, SD), np.float32)
    for c in range(NCORES):
        for s in range(STREAMS):
            lo = c * BLOCAL + s * W
            out = np.asarray(res.results[c][f"out{s}"], np.float16)
            out = out.reshape(SD, T + 1, W).transpose(2, 1, 0)  # [W, T+1, SD]
            states[lo : lo + W] = out.astype(np.float32)
    states[:, 0, :] = np.asarray(x0, np.float32)
    return states
